# revision 28
# baseline (speedup 1.0000x reference)
"""MinGRU (2-layer) Trainium2 Bass kernel — fp8 DoubleRow edition.

Problem: B=8, S=4096, D=H=1024.
  layer(inp, W, b): gh = inp @ W.T + b ; gate, hid = split(gh)
    z = sigmoid(gate); a = 1 - z = sigmoid(-gate)
    g = where(hid >= 0, hid + 0.5, sigmoid(hid)) = max(hid + 0.5, sigmoid(hid))
    h_t = a_t * h_{t-1} + z_t * g_t        (h_0 = 0.5)
  out = layer(layer(x, W0, b0), W1, b1)

Sharding: data-parallel over batch, one batch per NeuronCore (8 cores).

Per-core dataflow (batch b):
  - GEMMs run in fp8 e4m3 with MatmulPerfMode.DoubleRow (k=256 per matmul,
    0.5 cy/row): weights are host-scaled by 512 (keeps them in e4m3 normal
    range), gate weights negated so both sigmoids share one scale/bias.
  - biases ride a 5th k=4 DoubleRow matmul per PSUM group: each bias value
    512*(±b + 0.5) is residual-split into 4 fp8 rows whose sum carries
    ~1e-5 relative error; the +0.5*512 makes ps_hid = 512*(hid_b + 0.5).
  - per (feature-tile f, chunk-pair c2) a [128, 2048] PSUM tile holds
    [gate c0 | gate c1 | hid c0 | hid c1]; ONE fused ScalarE sigmoid over
    all 2048 cols yields [a0|a1|s0|s1] (scale 1/512, bias -0.5), and one
    ScalarE Copy descales the hid span to t = hid_b + 0.5 (bf16).
  - VectorE: m = max(t, s) (bf16 2x mode), bneg = (a-1)*m in place,
    then one tensor_tensor_scan over 1024 timesteps (fp32 state),
    chained across chunk-pairs via initial=prev[:, -1:].
  - layer-0 h is written by the scan directly as fp8 into SBUF-resident
    per-(chunk-pair, k-tile) buffers laid out as layer-1's DoubleRow rhs;
    no DRAM round-trip for h1.
  - layer-1 h is written bf16 and DMA'd to the (H, S) output; host
    transposes/upcasts.
  - engine balance (TimelineSim): ACT ~180us (fused sigmoid + descale
    Copy), DVE ~174us (a-1, max, mul, scan), PE ~144us, total ~231us
    vs 464us for the fp32r baseline (506us measured on HW).
  - measured on HW via the axon PJRT path: rel err 1.3e-2 (gate 2e-2).
"""
import sys

sys.path.insert(0, "/opt/trn_rl_repo")

import numpy as np
import ml_dtypes
from contextlib import ExitStack

from concourse import bacc, tile, mybir

dt = mybir.dt
Alu = mybir.AluOpType
Act = mybir.ActivationFunctionType
PM = mybir.MatmulPerfMode

B, S, D, H = 8, 4096, 1024, 1024
SC = 512                 # seq chunk (PSUM-bank-sized matmul N)
NC2 = 4                  # chunk-pairs of 1024 timesteps
NKT = 4                  # contraction tiles of 256 (DoubleRow)
NFB = 8                  # feature blocks of 128
ISC = 1.0 / 512.0        # descale applied in ScalarE ops

_cached = {}


def _build():
    nc = bacc.Bacc("TRN2", target_bir_lowering=False, debug=False, num_devices=8)

    # x packed [p, (kt, c, half, t)] fp8
    d_x = nc.dram_tensor("xpk", [128, NKT * 8 * 2 * SC], dt.float8e4,
                         kind="ExternalInput").ap()
    # weights packed [p, (kt, ot, half, m)] fp8 (gate negated, x512)
    d_w0 = nc.dram_tensor("w0pk", [128, NKT * 16 * 2 * 128], dt.float8e4,
                          kind="ExternalInput").ap()
    d_w1 = nc.dram_tensor("w1pk", [128, NKT * 16 * 2 * 128], dt.float8e4,
                          kind="ExternalInput").ap()
    # bias residual rows [p(2), (i, ot, m)] fp8
    d_b0 = nc.dram_tensor("b0pk", [2, 2 * 16 * 128], dt.float8e4,
                          kind="ExternalInput").ap()
    d_b1 = nc.dram_tensor("b1pk", [2, 2 * 16 * 128], dt.float8e4,
                          kind="ExternalInput").ap()
    d_out = nc.dram_tensor("outT", [H, S], dt.bfloat16, kind="ExternalOutput").ap()

    with tile.TileContext(nc) as tc, ExitStack() as ctx:
        cpool = ctx.enter_context(tc.tile_pool(name="const", bufs=1))
        wpool = ctx.enter_context(tc.tile_pool(name="w", bufs=1))
        xpool = ctx.enter_context(tc.tile_pool(name="x", bufs=2))
        hpool = ctx.enter_context(tc.tile_pool(name="h1", bufs=1))
        apool = ctx.enter_context(tc.tile_pool(name="as", bufs=6))
        tpool = ctx.enter_context(tc.tile_pool(name="t", bufs=6))
        upool = ctx.enter_context(tc.tile_pool(name="u", bufs=6))
        opool = ctx.enter_context(tc.tile_pool(name="ho", bufs=2))
        pspool = ctx.enter_context(tc.tile_pool(name="ps", bufs=2, space="PSUM"))

        t_b0 = cpool.tile([2, 2 * 16 * 128], dt.float8e4)
        nc.gpsimd.dma_start(t_b0[:], d_b0)
        t_b1 = cpool.tile([2, 2 * 16 * 128], dt.float8e4)
        nc.gpsimd.dma_start(t_b1[:], d_b1)
        ones = cpool.tile([2, 2 * SC], dt.float8e4)
        nc.gpsimd.memset(ones[:], 1.0)
        ones_v = ones[:].rearrange("p (i t) -> p i t", i=2)
        neghalf = cpool.tile([128, 1], dt.float32)
        nc.gpsimd.memset(neghalf[:], -0.5)

        t_w0 = wpool.tile([128, NKT * 16 * 2 * 128], dt.float8e4, name="w0t")
        t_w1 = wpool.tile([128, NKT * 16 * 2 * 128], dt.float8e4, name="w1t")

        def load_w(t_w, d_w, kt):
            seg = 16 * 2 * 128
            nc.sync.dma_start(t_w[:, seg * kt: seg * (kt + 1)],
                              d_w[:, seg * kt: seg * (kt + 1)])

        load_w(t_w0, d_w0, 0)

        # h1 SBUF-resident: one tile per (c2, kt), [p, (half, c, t)] fp8
        h1_tiles = [
            [hpool.tile([128, 2 * 1024], dt.float8e4, name=f"h1_{c2}_{kt}",
                        tag=f"h1_{c2}_{kt}", bufs=1) for kt in range(NKT)]
            for c2 in range(NC2)
        ]

        w0_v = t_w0[:].rearrange("p (kt ot i m) -> kt ot p i m", kt=NKT, ot=16, i=2)
        w1_v = t_w1[:].rearrange("p (kt ot i m) -> kt ot p i m", kt=NKT, ot=16, i=2)
        b0_v = t_b0[:].rearrange("p (i ot m) -> ot p i m", i=2, ot=16)
        b1_v = t_b1[:].rearrange("p (i ot m) -> ot p i m", i=2, ot=16)

        def do_layer(w_v, b_v, rhs_of, is_last, prefetch=None):
            carry = [None] * NFB
            for c2 in range(NC2):
                rhs_tiles = rhs_of(c2)
                if prefetch is not None:
                    prefetch(c2)
                for f in range(NFB):
                    ps = pspool.tile([128, 4 * SC], dt.float32, name="ps", tag="ps")
                    for q in range(4):
                        ot = f if q < 2 else 8 + f
                        cj = q % 2
                        reg = ps[:, SC * q: SC * (q + 1)]
                        nc.tensor.matmul(reg, b_v[ot], ones_v,
                                         start=True, stop=False, perf_mode=PM.DoubleRow)
                        for kt in range(NKT):
                            nc.tensor.matmul(reg, w_v[kt, ot], rhs_tiles(kt, cj),
                                             start=False, stop=(kt == NKT - 1),
                                             perf_mode=PM.DoubleRow)
                    # fused sigmoid: [a0|a1|s0|s1]
                    as_ = apool.tile([128, 4 * SC], dt.bfloat16, name="as", tag="as")
                    nc.scalar.activation(as_[:], ps[:], Act.Sigmoid,
                                         bias=neghalf[:], scale=ISC)
                    # u = a - 1 (4x-mode tensor_scalar)
                    u = upool.tile([128, 2 * SC], dt.bfloat16, name="u", tag="u")
                    nc.vector.tensor_scalar(u[:], as_[:, :2 * SC], 1.0, None,
                                            op0=Alu.subtract)
                    t = tpool.tile([128, 2 * SC], dt.bfloat16, name="t", tag="t")
                    if f < 7:
                        # t = hid_b + 0.5 (ScalarE descale), m = max(t, s)
                        nc.scalar.activation(t[:], ps[:, 2 * SC:], Act.Copy,
                                             bias=0.0, scale=ISC)
                        nc.vector.tensor_max(t[:], t[:], as_[:, 2 * SC:])
                    else:
                        # balance: every 8th tile computes m on VectorE
                        nc.vector.scalar_tensor_tensor(
                            t[:], ps[:, 2 * SC:], ISC, as_[:, 2 * SC:],
                            op0=Alu.mult, op1=Alu.max)
                    # bneg = (a - 1) * m   (2x-mode, in place onto t)
                    nc.vector.tensor_mul(t[:], u[:], t[:])
                    # recurrence h = a*h_prev - bneg over 1024 steps
                    if is_last:
                        ho = opool.tile([128, 2 * SC], dt.bfloat16,
                                        name=f"ho{f}", tag=f"ho{f}")
                        dest = ho[:]
                    else:
                        ht = h1_tiles[c2][f // 2]
                        dest = ht[:, (f % 2) * 1024: (f % 2 + 1) * 1024]
                    init = 0.5 if c2 == 0 else carry[f]
                    nc.vector.tensor_tensor_scan(dest, as_[:, :2 * SC], t[:], init,
                                                 op0=Alu.mult, op1=Alu.subtract)
                    carry[f] = dest[:, 2 * SC - 1: 2 * SC]
                    if is_last:
                        nc.sync.dma_start(
                            d_out[128 * f: 128 * (f + 1),
                                  1024 * c2: 1024 * (c2 + 1)], dest)

        # whole x resident: one tile per (kt, c2); c2=0 lands first so the
        # first PSUM groups aren't gated on the full 4MB
        x_tiles = [
            [xpool.tile([128, 2048], dt.float8e4, name=f"x{kt}_{c2}",
                        tag=f"x{kt}_{c2}", bufs=1) for c2 in range(NC2)]
            for kt in range(NKT)
        ]
        for kt in range(NKT):
            nc.sync.dma_start(x_tiles[kt][0][:], d_x[:, kt * 8192: kt * 8192 + 2048])

        def rhs_x(c2):
            def get(kt, cj):
                return x_tiles[kt][c2][:, 1024 * cj: 1024 * (cj + 1)].rearrange(
                    "p (i t) -> p i t", i=2)
            return get

        def rhs_h1(c2):
            def get(kt, cj):
                return h1_tiles[c2][kt][:].rearrange(
                    "p (i c t) -> c p i t", i=2, c=2)[cj]
            return get

        def prefetch_l0(c2):
            if c2 == 0:
                for kt in range(1, NKT):
                    load_w(t_w0, d_w0, kt)
                for cx in range(1, 4):
                    for kt in range(NKT):
                        base = kt * 8192 + cx * 2048
                        nc.sync.dma_start(x_tiles[kt][cx][:],
                                          d_x[:, base: base + 2048])
            else:
                load_w(t_w1, d_w1, c2 - 1)

        def prefetch_l0b(c2):
            if c2 == 0:
                load_w(t_w1, d_w1, NKT - 1)

        do_layer(w0_v, b0_v, rhs_x, is_last=False, prefetch=prefetch_l0)
        do_layer(w1_v, b1_v, rhs_h1, is_last=True, prefetch=prefetch_l0b)

    nc.compile()
    return nc


def _q8(v):
    return np.asarray(v, np.float32).astype(ml_dtypes.float8_e4m3)


def _pack_w(W):
    # [p, (kt, ot, half, m)]; gate rows negated; x512
    Ws = np.asarray(W, np.float32) * 512.0
    Ws[:H] *= -1.0
    # arr[ot, m, kt, i, p]
    a = Ws.reshape(16, 128, NKT, 2, 128)
    a = a.transpose(4, 2, 0, 3, 1)  # p, kt, ot, i, m
    return np.ascontiguousarray(_q8(a.reshape(128, NKT * 16 * 2 * 128)))


def _pack_b(b):
    v = np.asarray(b, np.float32) * 512.0
    v[:H] *= -1.0
    v = v + 256.0            # 512*(±b) + 512*0.5
    v = v.reshape(16, 128)   # [ot, m]
    # e4m3 max finite is ~240 but v ~ 256: halve the first row so every
    # residual row stays well inside range
    rows = []
    rem = v.copy()
    r = _q8(rem * 0.5)
    rows.append(r)
    rem = rem - r.astype(np.float32)
    for _ in range(3):
        r = _q8(rem)
        rows.append(r)
        rem = rem - r.astype(np.float32)
    assert np.isfinite(rows[0].astype(np.float32)).all()
    rows = np.stack(rows)    # [4, ot, m]; sum == v (to ~1e-5)
    # row j -> (p, i) with j = 2*i + p
    arr = np.zeros((2, 2, 16, 128), ml_dtypes.float8_e4m3)
    for j in range(4):
        arr[j % 2, j // 2] = rows[j]
    return np.ascontiguousarray(arr.reshape(2, 2 * 16 * 128))


def _pack_x(xb):
    # xb (S, D) -> [p, (kt, c, half, t)]
    a = np.asarray(xb, np.float32).reshape(8, SC, NKT, 2, 128)  # c, t, kt, i, p
    a = a.transpose(4, 2, 0, 3, 1)  # p, kt, c, i, t
    return np.ascontiguousarray(_q8(a.reshape(128, NKT * 8 * 2 * SC)))


def kernel(x, W0, b0, W1, b1):
    from concourse.bass_utils import run_bass_kernel_spmd

    if "nc" not in _cached:
        _cached["nc"] = _build()
    nc = _cached["nc"]

    x = np.asarray(x)
    shared = {
        "w0pk": _pack_w(W0), "w1pk": _pack_w(W1),
        "b0pk": _pack_b(b0), "b1pk": _pack_b(b1),
    }
    in_maps = []
    for b in range(B):
        m = dict(shared)
        m["xpk"] = _pack_x(x[b])
        in_maps.append(m)

    res = run_bass_kernel_spmd(nc, in_maps, core_ids=list(range(B)))
    out = np.empty((B, S, H), np.float32)
    for b in range(B):
        out[b] = res.results[b]["outT"].astype(np.float32).T
    return out


# revision 29
# speedup vs baseline: 1.0031x; 1.0031x over previous
"""MinGRU (2-layer) Trainium2 Bass kernel — fp8 DoubleRow edition.

Problem: B=8, S=4096, D=H=1024.
  layer(inp, W, b): gh = inp @ W.T + b ; gate, hid = split(gh)
    z = sigmoid(gate); a = 1 - z = sigmoid(-gate)
    g = where(hid >= 0, hid + 0.5, sigmoid(hid)) = max(hid + 0.5, sigmoid(hid))
    h_t = a_t * h_{t-1} + z_t * g_t        (h_0 = 0.5)
  out = layer(layer(x, W0, b0), W1, b1)

Sharding: data-parallel over batch, one batch per NeuronCore (8 cores).

Per-core dataflow (batch b):
  - GEMMs run in fp8 e4m3 with MatmulPerfMode.DoubleRow (k=256 per matmul,
    0.5 cy/row): weights are host-scaled by 512 (keeps them in e4m3 normal
    range), gate weights negated so both sigmoids share one scale/bias.
  - biases ride a 5th k=4 DoubleRow matmul per PSUM group: each bias value
    512*(±b + 0.5) is residual-split into 4 fp8 rows whose sum carries
    ~1e-5 relative error; the +0.5*512 makes ps_hid = 512*(hid_b + 0.5).
  - per (feature-tile f, chunk-pair c2) a [128, 2048] PSUM tile holds
    [gate c0 | gate c1 | hid c0 | hid c1]; ONE fused ScalarE sigmoid over
    all 2048 cols yields [a0|a1|s0|s1] (scale 1/512, bias -0.5), and one
    ScalarE Copy descales the hid span to t = hid_b + 0.5 (bf16).
  - VectorE: m = max(t, s) (bf16 2x mode), bneg = (a-1)*m in place,
    then one tensor_tensor_scan over 1024 timesteps (fp32 state),
    chained across chunk-pairs via initial=prev[:, -1:].
  - layer-0 h is written by the scan directly as fp8 into SBUF-resident
    per-(chunk-pair, k-tile) buffers laid out as layer-1's DoubleRow rhs;
    no DRAM round-trip for h1.
  - layer-1 h is written bf16 and DMA'd to the (H, S) output; host
    transposes/upcasts.
  - engine balance (TimelineSim): ACT ~180us (fused sigmoid + descale
    Copy), DVE ~174us (a-1, max, mul, scan), PE ~144us, total ~231us
    vs 464us for the fp32r baseline (506us measured on HW).
  - measured on HW via the axon PJRT path: rel err 1.3e-2 (gate 2e-2).
"""
import sys

sys.path.insert(0, "/opt/trn_rl_repo")

import numpy as np
import ml_dtypes
from contextlib import ExitStack

from concourse import bacc, tile, mybir

dt = mybir.dt
Alu = mybir.AluOpType
Act = mybir.ActivationFunctionType
PM = mybir.MatmulPerfMode

B, S, D, H = 8, 4096, 1024, 1024
SC = 512                 # seq chunk (PSUM-bank-sized matmul N)
NC2 = 4                  # chunk-pairs of 1024 timesteps
NKT = 4                  # contraction tiles of 256 (DoubleRow)
NFB = 8                  # feature blocks of 128
ISC = 1.0 / 512.0        # descale applied in ScalarE ops

_cached = {}


def _build():
    nc = bacc.Bacc("TRN2", target_bir_lowering=False, debug=False, num_devices=8)

    # x packed [p, (kt, c, half, t)] fp8
    d_x = nc.dram_tensor("xpk", [128, NKT * 8 * 2 * SC], dt.float8e4,
                         kind="ExternalInput").ap()
    # weights packed [p, (kt, ot, half, m)] fp8 (gate negated, x512)
    d_w0 = nc.dram_tensor("w0pk", [128, NKT * 16 * 2 * 128], dt.float8e4,
                          kind="ExternalInput").ap()
    d_w1 = nc.dram_tensor("w1pk", [128, NKT * 16 * 2 * 128], dt.float8e4,
                          kind="ExternalInput").ap()
    # bias residual rows [p(2), (i, ot, m)] fp8
    d_b0 = nc.dram_tensor("b0pk", [2, 2 * 16 * 128], dt.float8e4,
                          kind="ExternalInput").ap()
    d_b1 = nc.dram_tensor("b1pk", [2, 2 * 16 * 128], dt.float8e4,
                          kind="ExternalInput").ap()
    d_out = nc.dram_tensor("outT", [H, S], dt.bfloat16, kind="ExternalOutput").ap()

    with tile.TileContext(nc) as tc, ExitStack() as ctx:
        cpool = ctx.enter_context(tc.tile_pool(name="const", bufs=1))
        wpool = ctx.enter_context(tc.tile_pool(name="w", bufs=1))
        xpool = ctx.enter_context(tc.tile_pool(name="x", bufs=2))
        hpool = ctx.enter_context(tc.tile_pool(name="h1", bufs=1))
        apool = ctx.enter_context(tc.tile_pool(name="as", bufs=6))
        tpool = ctx.enter_context(tc.tile_pool(name="t", bufs=6))
        upool = ctx.enter_context(tc.tile_pool(name="u", bufs=6))
        opool = ctx.enter_context(tc.tile_pool(name="ho", bufs=3))
        pspool = ctx.enter_context(tc.tile_pool(name="ps", bufs=2, space="PSUM"))

        t_b0 = cpool.tile([2, 2 * 16 * 128], dt.float8e4)
        nc.gpsimd.dma_start(t_b0[:], d_b0)
        t_b1 = cpool.tile([2, 2 * 16 * 128], dt.float8e4)
        nc.gpsimd.dma_start(t_b1[:], d_b1)
        ones = cpool.tile([2, 2 * SC], dt.float8e4)
        nc.gpsimd.memset(ones[:], 1.0)
        ones_v = ones[:].rearrange("p (i t) -> p i t", i=2)
        neghalf = cpool.tile([128, 1], dt.float32)
        nc.gpsimd.memset(neghalf[:], -0.5)

        t_w0 = wpool.tile([128, NKT * 16 * 2 * 128], dt.float8e4, name="w0t")
        t_w1 = wpool.tile([128, NKT * 16 * 2 * 128], dt.float8e4, name="w1t")

        def load_w(t_w, d_w, kt):
            seg = 16 * 2 * 128
            nc.sync.dma_start(t_w[:, seg * kt: seg * (kt + 1)],
                              d_w[:, seg * kt: seg * (kt + 1)])

        load_w(t_w0, d_w0, 0)

        # h1 SBUF-resident: one tile per (c2, kt), [p, (half, c, t)] fp8
        h1_tiles = [
            [hpool.tile([128, 2 * 1024], dt.float8e4, name=f"h1_{c2}_{kt}",
                        tag=f"h1_{c2}_{kt}", bufs=1) for kt in range(NKT)]
            for c2 in range(NC2)
        ]

        w0_v = t_w0[:].rearrange("p (kt ot i m) -> kt ot p i m", kt=NKT, ot=16, i=2)
        w1_v = t_w1[:].rearrange("p (kt ot i m) -> kt ot p i m", kt=NKT, ot=16, i=2)
        b0_v = t_b0[:].rearrange("p (i ot m) -> ot p i m", i=2, ot=16)
        b1_v = t_b1[:].rearrange("p (i ot m) -> ot p i m", i=2, ot=16)

        def do_layer(w_v, b_v, rhs_of, is_last, prefetch=None):
            carry = [None] * NFB
            for c2 in range(NC2):
                rhs_tiles = rhs_of(c2)
                if prefetch is not None:
                    prefetch(c2)
                for f in range(NFB):
                    ps = pspool.tile([128, 4 * SC], dt.float32, name="ps", tag="ps")
                    for q in range(4):
                        ot = f if q < 2 else 8 + f
                        cj = q % 2
                        reg = ps[:, SC * q: SC * (q + 1)]
                        nc.tensor.matmul(reg, b_v[ot], ones_v,
                                         start=True, stop=False, perf_mode=PM.DoubleRow)
                        for kt in range(NKT):
                            nc.tensor.matmul(reg, w_v[kt, ot], rhs_tiles(kt, cj),
                                             start=False, stop=(kt == NKT - 1),
                                             perf_mode=PM.DoubleRow)
                    # fused sigmoid: [a0|a1|s0|s1]
                    as_ = apool.tile([128, 4 * SC], dt.bfloat16, name="as", tag="as")
                    nc.scalar.activation(as_[:], ps[:], Act.Sigmoid,
                                         bias=neghalf[:], scale=ISC)
                    # u = a - 1 (4x-mode tensor_scalar)
                    u = upool.tile([128, 2 * SC], dt.bfloat16, name="u", tag="u")
                    nc.vector.tensor_scalar(u[:], as_[:, :2 * SC], 1.0, None,
                                            op0=Alu.subtract)
                    t = tpool.tile([128, 2 * SC], dt.bfloat16, name="t", tag="t")
                    if f > 0:
                        # t = hid_b + 0.5 (ScalarE descale), m = max(t, s)
                        nc.scalar.activation(t[:], ps[:, 2 * SC:], Act.Copy,
                                             bias=0.0, scale=ISC)
                        nc.vector.tensor_max(t[:], t[:], as_[:, 2 * SC:])
                    else:
                        # balance: every 8th tile computes m on VectorE
                        nc.vector.scalar_tensor_tensor(
                            t[:], ps[:, 2 * SC:], ISC, as_[:, 2 * SC:],
                            op0=Alu.mult, op1=Alu.max)
                    # bneg = (a - 1) * m   (2x-mode, in place onto t)
                    nc.vector.tensor_mul(t[:], u[:], t[:])
                    # recurrence h = a*h_prev - bneg over 1024 steps
                    if is_last:
                        ho = opool.tile([128, 2 * SC], dt.bfloat16,
                                        name=f"ho{f}", tag=f"ho{f}")
                        dest = ho[:]
                    else:
                        ht = h1_tiles[c2][f // 2]
                        dest = ht[:, (f % 2) * 1024: (f % 2 + 1) * 1024]
                    init = 0.5 if c2 == 0 else carry[f]
                    nc.vector.tensor_tensor_scan(dest, as_[:, :2 * SC], t[:], init,
                                                 op0=Alu.mult, op1=Alu.subtract)
                    carry[f] = dest[:, 2 * SC - 1: 2 * SC]
                    if is_last:
                        nc.sync.dma_start(
                            d_out[128 * f: 128 * (f + 1),
                                  1024 * c2: 1024 * (c2 + 1)], dest)

        # whole x resident: one tile per (kt, c2); c2=0 lands first so the
        # first PSUM groups aren't gated on the full 4MB
        x_tiles = [
            [xpool.tile([128, 2048], dt.float8e4, name=f"x{kt}_{c2}",
                        tag=f"x{kt}_{c2}", bufs=1) for c2 in range(NC2)]
            for kt in range(NKT)
        ]
        for kt in range(NKT):
            nc.sync.dma_start(x_tiles[kt][0][:], d_x[:, kt * 8192: kt * 8192 + 2048])

        def rhs_x(c2):
            def get(kt, cj):
                return x_tiles[kt][c2][:, 1024 * cj: 1024 * (cj + 1)].rearrange(
                    "p (i t) -> p i t", i=2)
            return get

        def rhs_h1(c2):
            def get(kt, cj):
                return h1_tiles[c2][kt][:].rearrange(
                    "p (i c t) -> c p i t", i=2, c=2)[cj]
            return get

        def prefetch_l0(c2):
            if c2 == 0:
                for kt in range(1, NKT):
                    load_w(t_w0, d_w0, kt)
                for cx in range(1, 4):
                    for kt in range(NKT):
                        base = kt * 8192 + cx * 2048
                        nc.sync.dma_start(x_tiles[kt][cx][:],
                                          d_x[:, base: base + 2048])
            else:
                load_w(t_w1, d_w1, c2 - 1)

        def prefetch_l0b(c2):
            if c2 == 0:
                load_w(t_w1, d_w1, NKT - 1)

        do_layer(w0_v, b0_v, rhs_x, is_last=False, prefetch=prefetch_l0)
        do_layer(w1_v, b1_v, rhs_h1, is_last=True, prefetch=prefetch_l0b)

    nc.compile()
    return nc


def _q8(v):
    return np.asarray(v, np.float32).astype(ml_dtypes.float8_e4m3)


def _pack_w(W):
    # [p, (kt, ot, half, m)]; gate rows negated; x512
    Ws = np.asarray(W, np.float32) * 512.0
    Ws[:H] *= -1.0
    # arr[ot, m, kt, i, p]
    a = Ws.reshape(16, 128, NKT, 2, 128)
    a = a.transpose(4, 2, 0, 3, 1)  # p, kt, ot, i, m
    return np.ascontiguousarray(_q8(a.reshape(128, NKT * 16 * 2 * 128)))


def _pack_b(b):
    v = np.asarray(b, np.float32) * 512.0
    v[:H] *= -1.0
    v = v + 256.0            # 512*(±b) + 512*0.5
    v = v.reshape(16, 128)   # [ot, m]
    # e4m3 max finite is ~240 but v ~ 256: halve the first row so every
    # residual row stays well inside range
    rows = []
    rem = v.copy()
    r = _q8(rem * 0.5)
    rows.append(r)
    rem = rem - r.astype(np.float32)
    for _ in range(3):
        r = _q8(rem)
        rows.append(r)
        rem = rem - r.astype(np.float32)
    assert np.isfinite(rows[0].astype(np.float32)).all()
    rows = np.stack(rows)    # [4, ot, m]; sum == v (to ~1e-5)
    # row j -> (p, i) with j = 2*i + p
    arr = np.zeros((2, 2, 16, 128), ml_dtypes.float8_e4m3)
    for j in range(4):
        arr[j % 2, j // 2] = rows[j]
    return np.ascontiguousarray(arr.reshape(2, 2 * 16 * 128))


def _pack_x(xb):
    # xb (S, D) -> [p, (kt, c, half, t)]
    a = np.asarray(xb, np.float32).reshape(8, SC, NKT, 2, 128)  # c, t, kt, i, p
    a = a.transpose(4, 2, 0, 3, 1)  # p, kt, c, i, t
    return np.ascontiguousarray(_q8(a.reshape(128, NKT * 8 * 2 * SC)))


def kernel(x, W0, b0, W1, b1):
    from concourse.bass_utils import run_bass_kernel_spmd

    if "nc" not in _cached:
        _cached["nc"] = _build()
    nc = _cached["nc"]

    x = np.asarray(x)
    shared = {
        "w0pk": _pack_w(W0), "w1pk": _pack_w(W1),
        "b0pk": _pack_b(b0), "b1pk": _pack_b(b1),
    }
    in_maps = []
    for b in range(B):
        m = dict(shared)
        m["xpk"] = _pack_x(x[b])
        in_maps.append(m)

    res = run_bass_kernel_spmd(nc, in_maps, core_ids=list(range(B)))
    out = np.empty((B, S, H), np.float32)
    for b in range(B):
        out[b] = res.results[b]["outT"].astype(np.float32).T
    return out


# revision 30
# speedup vs baseline: 1.0988x; 1.0953x over previous
"""MinGRU (2-layer) Trainium2 Bass kernel — fp8 DoubleRow edition.

Problem: B=8, S=4096, D=H=1024.
  layer(inp, W, b): gh = inp @ W.T + b ; gate, hid = split(gh)
    z = sigmoid(gate); a = 1 - z = sigmoid(-gate)
    g = where(hid >= 0, hid + 0.5, sigmoid(hid)) = max(hid + 0.5, sigmoid(hid))
    h_t = a_t * h_{t-1} + z_t * g_t        (h_0 = 0.5)
  out = layer(layer(x, W0, b0), W1, b1)

Sharding: data-parallel over batch, one batch per NeuronCore (8 cores).

Per-core dataflow (batch b):
  - GEMMs run in fp8 e4m3 with MatmulPerfMode.DoubleRow (k=256 per matmul,
    0.5 cy/row): weights are host-scaled by 512 (keeps them in e4m3 normal
    range), gate weights negated so both sigmoids share one scale/bias.
  - biases ride a 5th k=4 DoubleRow matmul per PSUM group: each bias value
    512*(±b + 0.5) is residual-split into 4 fp8 rows whose sum carries
    ~1e-5 relative error; the +0.5*512 makes ps_hid = 512*(hid_b + 0.5).
  - per (feature-tile f, chunk-pair c2) a [128, 2048] PSUM tile holds
    [gate c0 | gate c1 | hid c0 | hid c1]; ONE fused ScalarE sigmoid over
    all 2048 cols yields [a0|a1|s0|s1] (scale 1/512, bias -0.5), and one
    ScalarE Copy descales the hid span to t = hid_b + 0.5 (bf16).
  - VectorE: m = max(t, s) (bf16 2x mode), bneg = (a-1)*m in place,
    then one tensor_tensor_scan over 1024 timesteps (fp32 state),
    chained across chunk-pairs via initial=prev[:, -1:].
  - layer-0 h is written by the scan directly as fp8 into SBUF-resident
    per-(chunk-pair, k-tile) buffers laid out as layer-1's DoubleRow rhs;
    no DRAM round-trip for h1.
  - layer-1 h is written bf16 and DMA'd to the (H, S) output; host
    transposes/upcasts.
  - engine balance (TimelineSim): ACT ~180us (fused sigmoid + descale
    Copy), DVE ~174us (a-1, max, mul, scan), PE ~144us, total ~231us
    vs 464us for the fp32r baseline (506us measured on HW).
  - measured on HW via the axon PJRT path: rel err 1.3e-2 (gate 2e-2).
"""
import sys

sys.path.insert(0, "/opt/trn_rl_repo")

import numpy as np
import ml_dtypes
from contextlib import ExitStack

from concourse import bacc, tile, mybir

dt = mybir.dt
Alu = mybir.AluOpType
Act = mybir.ActivationFunctionType
PM = mybir.MatmulPerfMode

B, S, D, H = 8, 4096, 1024, 1024
SC = 512                 # seq chunk (PSUM-bank-sized matmul N)
NC2 = 4                  # chunk-pairs of 1024 timesteps
NKT = 4                  # contraction tiles of 256 (DoubleRow)
NFB = 8                  # feature blocks of 128
ISC = 1.0 / 512.0        # descale applied in ScalarE ops

_cached = {}


def _build():
    nc = bacc.Bacc("TRN2", target_bir_lowering=False, debug=False, num_devices=8)

    # x packed [p, (kt, c, half, t)] fp8
    d_x = nc.dram_tensor("xpk", [128, NKT * 8 * 2 * SC], dt.float8e4,
                         kind="ExternalInput").ap()
    # weights packed [p, (kt, ot, half, m)] fp8 (gate negated, x512)
    d_w0 = nc.dram_tensor("w0pk", [128, NKT * 16 * 2 * 128], dt.float8e4,
                          kind="ExternalInput").ap()
    d_w1 = nc.dram_tensor("w1pk", [128, NKT * 16 * 2 * 128], dt.float8e4,
                          kind="ExternalInput").ap()
    # bias residual rows [p(2), (i, ot, m)] fp8
    d_b0 = nc.dram_tensor("b0pk", [2, 2 * 16 * 128], dt.float8e4,
                          kind="ExternalInput").ap()
    d_b1 = nc.dram_tensor("b1pk", [2, 2 * 16 * 128], dt.float8e4,
                          kind="ExternalInput").ap()
    d_out = nc.dram_tensor("outT", [H, S], dt.bfloat16, kind="ExternalOutput").ap()

    with tile.TileContext(nc) as tc, ExitStack() as ctx:
        cpool = ctx.enter_context(tc.tile_pool(name="const", bufs=1))
        wpool = ctx.enter_context(tc.tile_pool(name="w", bufs=1))
        xpool = ctx.enter_context(tc.tile_pool(name="x", bufs=2))
        hpool = ctx.enter_context(tc.tile_pool(name="h1", bufs=1))
        apool = ctx.enter_context(tc.tile_pool(name="as", bufs=6))
        tpool = ctx.enter_context(tc.tile_pool(name="t", bufs=6))
        upool = ctx.enter_context(tc.tile_pool(name="u", bufs=6))
        opool = ctx.enter_context(tc.tile_pool(name="ho", bufs=3))
        pspool = ctx.enter_context(tc.tile_pool(name="ps", bufs=2, space="PSUM"))

        t_b0 = cpool.tile([2, 2 * 16 * 128], dt.float8e4)
        nc.gpsimd.dma_start(t_b0[:], d_b0)
        t_b1 = cpool.tile([2, 2 * 16 * 128], dt.float8e4)
        nc.gpsimd.dma_start(t_b1[:], d_b1)
        ones = cpool.tile([2, 2 * SC], dt.float8e4)
        nc.gpsimd.memset(ones[:], 1.0)
        ones_v = ones[:].rearrange("p (i t) -> p i t", i=2)
        neghalf = cpool.tile([128, 1], dt.float32)
        nc.gpsimd.memset(neghalf[:], -0.5)

        t_w0 = wpool.tile([128, NKT * 16 * 2 * 128], dt.float8e4, name="w0t")
        t_w1 = wpool.tile([128, NKT * 16 * 2 * 128], dt.float8e4, name="w1t")

        def load_w(t_w, d_w, kt):
            seg = 16 * 2 * 128
            nc.sync.dma_start(t_w[:, seg * kt: seg * (kt + 1)],
                              d_w[:, seg * kt: seg * (kt + 1)])

        load_w(t_w0, d_w0, 0)

        # h1 SBUF-resident: one tile per (c2, kt), [p, (half, c, t)] fp8
        h1_tiles = [
            [hpool.tile([128, 2 * 1024], dt.float8e4, name=f"h1_{c2}_{kt}",
                        tag=f"h1_{c2}_{kt}", bufs=1) for kt in range(NKT)]
            for c2 in range(NC2)
        ]

        w0_v = t_w0[:].rearrange("p (kt ot i m) -> kt ot p i m", kt=NKT, ot=16, i=2)
        w1_v = t_w1[:].rearrange("p (kt ot i m) -> kt ot p i m", kt=NKT, ot=16, i=2)
        b0_v = t_b0[:].rearrange("p (i ot m) -> ot p i m", i=2, ot=16)
        b1_v = t_b1[:].rearrange("p (i ot m) -> ot p i m", i=2, ot=16)

        def do_layer(w_v, b_v, rhs_of, is_last, prefetch=None):
            carry = [None] * NFB
            for c2 in range(NC2):
                rhs_tiles = rhs_of(c2)
                if prefetch is not None:
                    prefetch(c2)
                for f in range(NFB):
                    ps = pspool.tile([128, 4 * SC], dt.float32, name="ps", tag="ps")
                    for q in range(4):
                        ot = f if q < 2 else 8 + f
                        cj = q % 2
                        reg = ps[:, SC * q: SC * (q + 1)]
                        nc.tensor.matmul(reg, b_v[ot], ones_v,
                                         start=True, stop=False, perf_mode=PM.DoubleRow)
                        for kt in range(NKT):
                            nc.tensor.matmul(reg, w_v[kt, ot], rhs_tiles(kt, cj),
                                             start=False, stop=(kt == NKT - 1),
                                             perf_mode=PM.DoubleRow)
                    # fused sigmoid: [a0|a1|s0|s1]
                    as_ = apool.tile([128, 4 * SC], dt.bfloat16, name="as", tag="as")
                    nc.scalar.activation(as_[:], ps[:], Act.Sigmoid,
                                         bias=neghalf[:], scale=ISC)
                    # u = a - 1 on the idle Pool engine (ready-early leaf op)
                    u = upool.tile([128, 2 * SC], dt.bfloat16, name="u", tag="u")
                    nc.gpsimd.tensor_scalar(u[:], as_[:, :2 * SC], 1.0, None,
                                            op0=Alu.subtract)
                    t = tpool.tile([128, 2 * SC], dt.bfloat16, name="t", tag="t")
                    if f % 2 == 1:
                        # t = hid_b + 0.5 (ScalarE descale), m = max(t, s)
                        nc.scalar.activation(t[:], ps[:, 2 * SC:], Act.Copy,
                                             bias=0.0, scale=ISC)
                        nc.vector.tensor_max(t[:], t[:], as_[:, 2 * SC:])
                    else:
                        # balance: every 8th tile computes m on VectorE
                        nc.vector.scalar_tensor_tensor(
                            t[:], ps[:, 2 * SC:], ISC, as_[:, 2 * SC:],
                            op0=Alu.mult, op1=Alu.max)
                    # bneg = (a - 1) * m   (2x-mode, in place onto t)
                    nc.vector.tensor_mul(t[:], u[:], t[:])
                    # recurrence h = a*h_prev - bneg over 1024 steps
                    if is_last:
                        ho = opool.tile([128, 2 * SC], dt.bfloat16,
                                        name=f"ho{f}", tag=f"ho{f}")
                        dest = ho[:]
                    else:
                        ht = h1_tiles[c2][f // 2]
                        dest = ht[:, (f % 2) * 1024: (f % 2 + 1) * 1024]
                    init = 0.5 if c2 == 0 else carry[f]
                    nc.vector.tensor_tensor_scan(dest, as_[:, :2 * SC], t[:], init,
                                                 op0=Alu.mult, op1=Alu.subtract)
                    carry[f] = dest[:, 2 * SC - 1: 2 * SC]
                    if is_last:
                        nc.sync.dma_start(
                            d_out[128 * f: 128 * (f + 1),
                                  1024 * c2: 1024 * (c2 + 1)], dest)

        # whole x resident: one tile per (kt, c2); c2=0 lands first so the
        # first PSUM groups aren't gated on the full 4MB
        x_tiles = [
            [xpool.tile([128, 2048], dt.float8e4, name=f"x{kt}_{c2}",
                        tag=f"x{kt}_{c2}", bufs=1) for c2 in range(NC2)]
            for kt in range(NKT)
        ]
        for kt in range(NKT):
            nc.sync.dma_start(x_tiles[kt][0][:], d_x[:, kt * 8192: kt * 8192 + 2048])

        def rhs_x(c2):
            def get(kt, cj):
                return x_tiles[kt][c2][:, 1024 * cj: 1024 * (cj + 1)].rearrange(
                    "p (i t) -> p i t", i=2)
            return get

        def rhs_h1(c2):
            def get(kt, cj):
                return h1_tiles[c2][kt][:].rearrange(
                    "p (i c t) -> c p i t", i=2, c=2)[cj]
            return get

        def prefetch_l0(c2):
            if c2 == 0:
                for kt in range(1, NKT):
                    load_w(t_w0, d_w0, kt)
                for cx in range(1, 4):
                    for kt in range(NKT):
                        base = kt * 8192 + cx * 2048
                        nc.sync.dma_start(x_tiles[kt][cx][:],
                                          d_x[:, base: base + 2048])
            else:
                load_w(t_w1, d_w1, c2 - 1)

        def prefetch_l0b(c2):
            if c2 == 0:
                load_w(t_w1, d_w1, NKT - 1)

        do_layer(w0_v, b0_v, rhs_x, is_last=False, prefetch=prefetch_l0)
        do_layer(w1_v, b1_v, rhs_h1, is_last=True, prefetch=prefetch_l0b)

    nc.compile()
    return nc


def _q8(v):
    return np.asarray(v, np.float32).astype(ml_dtypes.float8_e4m3)


def _pack_w(W):
    # [p, (kt, ot, half, m)]; gate rows negated; x512
    Ws = np.asarray(W, np.float32) * 512.0
    Ws[:H] *= -1.0
    # arr[ot, m, kt, i, p]
    a = Ws.reshape(16, 128, NKT, 2, 128)
    a = a.transpose(4, 2, 0, 3, 1)  # p, kt, ot, i, m
    return np.ascontiguousarray(_q8(a.reshape(128, NKT * 16 * 2 * 128)))


def _pack_b(b):
    v = np.asarray(b, np.float32) * 512.0
    v[:H] *= -1.0
    v = v + 256.0            # 512*(±b) + 512*0.5
    v = v.reshape(16, 128)   # [ot, m]
    # e4m3 max finite is ~240 but v ~ 256: halve the first row so every
    # residual row stays well inside range
    rows = []
    rem = v.copy()
    r = _q8(rem * 0.5)
    rows.append(r)
    rem = rem - r.astype(np.float32)
    for _ in range(3):
        r = _q8(rem)
        rows.append(r)
        rem = rem - r.astype(np.float32)
    assert np.isfinite(rows[0].astype(np.float32)).all()
    rows = np.stack(rows)    # [4, ot, m]; sum == v (to ~1e-5)
    # row j -> (p, i) with j = 2*i + p
    arr = np.zeros((2, 2, 16, 128), ml_dtypes.float8_e4m3)
    for j in range(4):
        arr[j % 2, j // 2] = rows[j]
    return np.ascontiguousarray(arr.reshape(2, 2 * 16 * 128))


def _pack_x(xb):
    # xb (S, D) -> [p, (kt, c, half, t)]
    a = np.asarray(xb, np.float32).reshape(8, SC, NKT, 2, 128)  # c, t, kt, i, p
    a = a.transpose(4, 2, 0, 3, 1)  # p, kt, c, i, t
    return np.ascontiguousarray(_q8(a.reshape(128, NKT * 8 * 2 * SC)))


def kernel(x, W0, b0, W1, b1):
    from concourse.bass_utils import run_bass_kernel_spmd

    if "nc" not in _cached:
        _cached["nc"] = _build()
    nc = _cached["nc"]

    x = np.asarray(x)
    shared = {
        "w0pk": _pack_w(W0), "w1pk": _pack_w(W1),
        "b0pk": _pack_b(b0), "b1pk": _pack_b(b1),
    }
    in_maps = []
    for b in range(B):
        m = dict(shared)
        m["xpk"] = _pack_x(x[b])
        in_maps.append(m)

    res = run_bass_kernel_spmd(nc, in_maps, core_ids=list(range(B)))
    out = np.empty((B, S, H), np.float32)
    for b in range(B):
        out[b] = res.results[b]["outT"].astype(np.float32).T
    return out


# revision 33
# speedup vs baseline: 1.0995x; 1.0006x over previous
"""MinGRU (2-layer) Trainium2 Bass kernel — fp8 DoubleRow edition.

Problem: B=8, S=4096, D=H=1024.
  layer(inp, W, b): gh = inp @ W.T + b ; gate, hid = split(gh)
    z = sigmoid(gate); a = 1 - z = sigmoid(-gate)
    g = where(hid >= 0, hid + 0.5, sigmoid(hid)) = max(hid + 0.5, sigmoid(hid))
    h_t = a_t * h_{t-1} + z_t * g_t        (h_0 = 0.5)
  out = layer(layer(x, W0, b0), W1, b1)

Sharding: data-parallel over batch, one batch per NeuronCore (8 cores).

Per-core dataflow (batch b):
  - GEMMs run in fp8 e4m3 with MatmulPerfMode.DoubleRow (k=256 per matmul,
    0.5 cy/row): weights are host-scaled by 512 (keeps them in e4m3 normal
    range), gate weights negated so both sigmoids share one scale/bias.
  - biases ride a 5th k=4 DoubleRow matmul per PSUM group: each bias value
    512*(±b + 0.5) is residual-split into 4 fp8 rows whose sum carries
    ~1e-5 relative error; the +0.5*512 makes ps_hid = 512*(hid_b + 0.5).
  - per (feature-tile f, chunk-pair c2) a [128, 2048] PSUM tile holds
    [gate c0 | gate c1 | hid c0 | hid c1]; ONE fused ScalarE sigmoid over
    all 2048 cols yields [a0|a1|s0|s1] (scale 1/512, bias -0.5), and one
    ScalarE Copy descales the hid span to t = hid_b + 0.5 (bf16).
  - u = a-1 runs on the otherwise-idle Pool (GPSIMD) engine — it is the
    one leaf op whose input (the sigmoid) is ready when Pool dequeues it;
    mid-chain ops on Pool serialize its in-order queue and lose.
  - VectorE: m = max(t, s) (bf16 2x mode) for odd feature tiles (ScalarE
    Copy descales), or a fused descale+max straight from PSUM for even
    ones (no fast mode, but it skips the ScalarE op — 4:4 alternation
    balances ACT 156us / DVE 167us / Pool 100us); then bneg = u*m and one
    tensor_tensor_scan over 1024 timesteps (fp32 state), chained across
    chunk-pairs via initial=prev[:, -1:].
  - layer-0 h is written by the scan directly as fp8 into SBUF-resident
    per-(chunk-pair, k-tile) buffers laid out as layer-1's DoubleRow rhs;
    no DRAM round-trip for h1.
  - layer-1 h is written bf16 and DMA'd to the (H, S) output; host
    transposes/upcasts.
  - TimelineSim total ~210us vs 464us for the fp32r baseline (which
    measured 506us on HW).
  - measured on HW via the axon PJRT path: rel err 1.318e-2 (gate 2e-2).
"""
import sys

sys.path.insert(0, "/opt/trn_rl_repo")

import numpy as np
import ml_dtypes
from contextlib import ExitStack

from concourse import bacc, tile, mybir

dt = mybir.dt
Alu = mybir.AluOpType
Act = mybir.ActivationFunctionType
PM = mybir.MatmulPerfMode

B, S, D, H = 8, 4096, 1024, 1024
SC = 512                 # seq chunk (PSUM-bank-sized matmul N)
NC2 = 4                  # chunk-pairs of 1024 timesteps
NKT = 4                  # contraction tiles of 256 (DoubleRow)
NFB = 8                  # feature blocks of 128
ISC = 1.0 / 512.0        # descale applied in ScalarE ops

_cached = {}


def _build():
    nc = bacc.Bacc("TRN2", target_bir_lowering=False, debug=False, num_devices=8)

    # x packed [p, (kt, c, half, t)] fp8
    d_x = nc.dram_tensor("xpk", [128, NKT * 8 * 2 * SC], dt.float8e4,
                         kind="ExternalInput").ap()
    # weights packed [p, (kt, ot, half, m)] fp8 (gate negated, x512)
    d_w0 = nc.dram_tensor("w0pk", [128, NKT * 16 * 2 * 128], dt.float8e4,
                          kind="ExternalInput").ap()
    d_w1 = nc.dram_tensor("w1pk", [128, NKT * 16 * 2 * 128], dt.float8e4,
                          kind="ExternalInput").ap()
    # bias residual rows [p(2), (i, ot, m)] fp8
    d_b0 = nc.dram_tensor("b0pk", [2, 2 * 16 * 128], dt.float8e4,
                          kind="ExternalInput").ap()
    d_b1 = nc.dram_tensor("b1pk", [2, 2 * 16 * 128], dt.float8e4,
                          kind="ExternalInput").ap()
    d_out = nc.dram_tensor("outT", [H, S], dt.bfloat16, kind="ExternalOutput").ap()

    with tile.TileContext(nc) as tc, ExitStack() as ctx:
        cpool = ctx.enter_context(tc.tile_pool(name="const", bufs=1))
        wpool = ctx.enter_context(tc.tile_pool(name="w", bufs=1))
        xpool = ctx.enter_context(tc.tile_pool(name="x", bufs=2))
        hpool = ctx.enter_context(tc.tile_pool(name="h1", bufs=1))
        apool = ctx.enter_context(tc.tile_pool(name="as", bufs=8))
        tpool = ctx.enter_context(tc.tile_pool(name="t", bufs=8))
        upool = ctx.enter_context(tc.tile_pool(name="u", bufs=8))
        opool = ctx.enter_context(tc.tile_pool(name="ho", bufs=2))
        pspool = ctx.enter_context(tc.tile_pool(name="ps", bufs=2, space="PSUM"))

        t_b0 = cpool.tile([2, 2 * 16 * 128], dt.float8e4)
        nc.gpsimd.dma_start(t_b0[:], d_b0)
        t_b1 = cpool.tile([2, 2 * 16 * 128], dt.float8e4)
        nc.gpsimd.dma_start(t_b1[:], d_b1)
        ones = cpool.tile([2, 2 * SC], dt.float8e4)
        nc.gpsimd.memset(ones[:], 1.0)
        ones_v = ones[:].rearrange("p (i t) -> p i t", i=2)
        neghalf = cpool.tile([128, 1], dt.float32)
        nc.gpsimd.memset(neghalf[:], -0.5)

        t_w0 = wpool.tile([128, NKT * 16 * 2 * 128], dt.float8e4, name="w0t")
        t_w1 = wpool.tile([128, NKT * 16 * 2 * 128], dt.float8e4, name="w1t")

        def load_w(t_w, d_w, kt):
            seg = 16 * 2 * 128
            nc.sync.dma_start(t_w[:, seg * kt: seg * (kt + 1)],
                              d_w[:, seg * kt: seg * (kt + 1)])

        load_w(t_w0, d_w0, 0)

        # h1 SBUF-resident: one tile per (c2, kt), [p, (half, c, t)] fp8
        h1_tiles = [
            [hpool.tile([128, 2 * 1024], dt.float8e4, name=f"h1_{c2}_{kt}",
                        tag=f"h1_{c2}_{kt}", bufs=1) for kt in range(NKT)]
            for c2 in range(NC2)
        ]

        w0_v = t_w0[:].rearrange("p (kt ot i m) -> kt ot p i m", kt=NKT, ot=16, i=2)
        w1_v = t_w1[:].rearrange("p (kt ot i m) -> kt ot p i m", kt=NKT, ot=16, i=2)
        b0_v = t_b0[:].rearrange("p (i ot m) -> ot p i m", i=2, ot=16)
        b1_v = t_b1[:].rearrange("p (i ot m) -> ot p i m", i=2, ot=16)

        def do_layer(w_v, b_v, rhs_of, is_last, prefetch=None):
            carry = [None] * NFB
            for c2 in range(NC2):
                rhs_tiles = rhs_of(c2)
                if prefetch is not None:
                    prefetch(c2)
                for f in range(NFB):
                    ps = pspool.tile([128, 4 * SC], dt.float32, name="ps", tag="ps")
                    for q in range(4):
                        ot = f if q < 2 else 8 + f
                        cj = q % 2
                        reg = ps[:, SC * q: SC * (q + 1)]
                        nc.tensor.matmul(reg, b_v[ot], ones_v,
                                         start=True, stop=False, perf_mode=PM.DoubleRow)
                        for kt in range(NKT):
                            nc.tensor.matmul(reg, w_v[kt, ot], rhs_tiles(kt, cj),
                                             start=False, stop=(kt == NKT - 1),
                                             perf_mode=PM.DoubleRow)
                    # fused sigmoid: [a0|a1|s0|s1]
                    as_ = apool.tile([128, 4 * SC], dt.bfloat16, name="as", tag="as")
                    nc.scalar.activation(as_[:], ps[:], Act.Sigmoid,
                                         bias=neghalf[:], scale=ISC)
                    # u = a - 1 on the idle Pool engine (ready-early leaf op)
                    u = upool.tile([128, 2 * SC], dt.bfloat16, name="u", tag="u")
                    nc.gpsimd.tensor_scalar(u[:], as_[:, :2 * SC], 1.0, None,
                                            op0=Alu.subtract)
                    t = tpool.tile([128, 2 * SC], dt.bfloat16, name="t", tag="t")
                    if f % 2 == 1:
                        # t = hid_b + 0.5 (ScalarE descale), m = max(t, s)
                        nc.scalar.activation(t[:], ps[:, 2 * SC:], Act.Copy,
                                             bias=0.0, scale=ISC)
                        nc.vector.tensor_max(t[:], t[:], as_[:, 2 * SC:])
                    else:
                        # balance: every 8th tile computes m on VectorE
                        nc.vector.scalar_tensor_tensor(
                            t[:], ps[:, 2 * SC:], ISC, as_[:, 2 * SC:],
                            op0=Alu.mult, op1=Alu.max)
                    # bneg = (a - 1) * m   (2x-mode, in place onto t)
                    nc.vector.tensor_mul(t[:], u[:], t[:])
                    # recurrence h = a*h_prev - bneg over 1024 steps
                    if is_last:
                        ho = opool.tile([128, 2 * SC], dt.bfloat16,
                                        name=f"ho{f}", tag=f"ho{f}")
                        dest = ho[:]
                    else:
                        ht = h1_tiles[c2][f // 2]
                        dest = ht[:, (f % 2) * 1024: (f % 2 + 1) * 1024]
                    init = 0.5 if c2 == 0 else carry[f]
                    nc.vector.tensor_tensor_scan(dest, as_[:, :2 * SC], t[:], init,
                                                 op0=Alu.mult, op1=Alu.subtract)
                    carry[f] = dest[:, 2 * SC - 1: 2 * SC]
                    if is_last:
                        nc.sync.dma_start(
                            d_out[128 * f: 128 * (f + 1),
                                  1024 * c2: 1024 * (c2 + 1)], dest)

        # whole x resident: one tile per (kt, c2); c2=0 lands first so the
        # first PSUM groups aren't gated on the full 4MB
        x_tiles = [
            [xpool.tile([128, 2048], dt.float8e4, name=f"x{kt}_{c2}",
                        tag=f"x{kt}_{c2}", bufs=1) for c2 in range(NC2)]
            for kt in range(NKT)
        ]
        for kt in range(NKT):
            nc.sync.dma_start(x_tiles[kt][0][:], d_x[:, kt * 8192: kt * 8192 + 2048])

        def rhs_x(c2):
            def get(kt, cj):
                return x_tiles[kt][c2][:, 1024 * cj: 1024 * (cj + 1)].rearrange(
                    "p (i t) -> p i t", i=2)
            return get

        def rhs_h1(c2):
            def get(kt, cj):
                return h1_tiles[c2][kt][:].rearrange(
                    "p (i c t) -> c p i t", i=2, c=2)[cj]
            return get

        def prefetch_l0(c2):
            if c2 == 0:
                for kt in range(1, NKT):
                    load_w(t_w0, d_w0, kt)
                for cx in range(1, 4):
                    for kt in range(NKT):
                        base = kt * 8192 + cx * 2048
                        nc.sync.dma_start(x_tiles[kt][cx][:],
                                          d_x[:, base: base + 2048])
            else:
                load_w(t_w1, d_w1, c2 - 1)

        def prefetch_l0b(c2):
            if c2 == 0:
                load_w(t_w1, d_w1, NKT - 1)

        do_layer(w0_v, b0_v, rhs_x, is_last=False, prefetch=prefetch_l0)
        do_layer(w1_v, b1_v, rhs_h1, is_last=True, prefetch=prefetch_l0b)

    nc.compile()
    return nc


def _q8(v):
    return np.asarray(v, np.float32).astype(ml_dtypes.float8_e4m3)


def _pack_w(W):
    # [p, (kt, ot, half, m)]; gate rows negated; x512
    Ws = np.asarray(W, np.float32) * 512.0
    Ws[:H] *= -1.0
    # arr[ot, m, kt, i, p]
    a = Ws.reshape(16, 128, NKT, 2, 128)
    a = a.transpose(4, 2, 0, 3, 1)  # p, kt, ot, i, m
    return np.ascontiguousarray(_q8(a.reshape(128, NKT * 16 * 2 * 128)))


def _pack_b(b):
    v = np.asarray(b, np.float32) * 512.0
    v[:H] *= -1.0
    v = v + 256.0            # 512*(±b) + 512*0.5
    v = v.reshape(16, 128)   # [ot, m]
    # e4m3 max finite is ~240 but v ~ 256: halve the first row so every
    # residual row stays well inside range
    rows = []
    rem = v.copy()
    r = _q8(rem * 0.5)
    rows.append(r)
    rem = rem - r.astype(np.float32)
    for _ in range(3):
        r = _q8(rem)
        rows.append(r)
        rem = rem - r.astype(np.float32)
    assert np.isfinite(rows[0].astype(np.float32)).all()
    rows = np.stack(rows)    # [4, ot, m]; sum == v (to ~1e-5)
    # row j -> (p, i) with j = 2*i + p
    arr = np.zeros((2, 2, 16, 128), ml_dtypes.float8_e4m3)
    for j in range(4):
        arr[j % 2, j // 2] = rows[j]
    return np.ascontiguousarray(arr.reshape(2, 2 * 16 * 128))


def _pack_x(xb):
    # xb (S, D) -> [p, (kt, c, half, t)]
    a = np.asarray(xb, np.float32).reshape(8, SC, NKT, 2, 128)  # c, t, kt, i, p
    a = a.transpose(4, 2, 0, 3, 1)  # p, kt, c, i, t
    return np.ascontiguousarray(_q8(a.reshape(128, NKT * 8 * 2 * SC)))


def kernel(x, W0, b0, W1, b1):
    from concourse.bass_utils import run_bass_kernel_spmd

    if "nc" not in _cached:
        _cached["nc"] = _build()
    nc = _cached["nc"]

    x = np.asarray(x)
    shared = {
        "w0pk": _pack_w(W0), "w1pk": _pack_w(W1),
        "b0pk": _pack_b(b0), "b1pk": _pack_b(b1),
    }
    in_maps = []
    for b in range(B):
        m = dict(shared)
        m["xpk"] = _pack_x(x[b])
        in_maps.append(m)

    res = run_bass_kernel_spmd(nc, in_maps, core_ids=list(range(B)))
    out = np.empty((B, S, H), np.float32)
    for b in range(B):
        out[b] = res.results[b]["outT"].astype(np.float32).T
    return out
